# revision 1
# baseline (speedup 1.0000x reference)
"""Trainium2 Bass kernel for the 2-layer sMGU RNN (nn_RVTDSMGU).

Key changes vs v1 (which serialized the two layers' scans per-engine):
  - The two layers' step chains are FUSED: one sigmoid/tanh/mul per step
    covers both layers (layer1 runs one chunk behind layer0, so their
    per-step work is independent), halving the serial latency chain.
  - Split trick: Uf @ h' = Uf @ d + Uf @ m (h' = d + m), so the next
    step's f-gate matmuls wait only on m = f*tanh, dropping the final
    h'-add from the cross-engine critical path.
  - Layer0 -> layer1 history handoff stays in SBUF (ping-pong tiles,
    loop unrolled x2) instead of a DRAM round-trip.
  - x-side weights/windows in bf16 (faster batched matmuls).

Sharding: data-parallel over batch, B=64 -> 8 cores x 8 samples.

Performance notes (measured 2026-08-09, axon-tunneled trn2):
  ~3.2-3.5 ms device time vs 9.99 ms baseline; rel_err 3.621e-03,
  bitwise deterministic. Per-step-pair latency ~1.5-1.9 us is the
  dependency floor of the MGU recurrence: six serial cross-engine hops
  (PE accum -> sigmoid -> mul -> PE accum -> tanh -> mul) whose
  latencies are fixed (PE PSUM drain ~173 ns, ACT dependent latency
  ~430 ns, DVE ~200 ns, semaphores ~40-60 ns/hop). Both layers share
  every hop; batched work runs in engine idle windows.

Hardware constraints discovered (violating these corrupts results):
  - GPSIMD cannot access PSUM (BIR verifier rejects).
  - A start=True matmul resets PSUM accumulation state beyond its own
    address range; partial-range openers need an explicit sync dep on
    the previous chunk's last reader of the bank (see open_bank).
  - Reading a fresh pool-tile allocation's memory expecting the prior
    allocation's data is unsynchronized (races) - carry cross-chunk
    state only through persistent tiles (hcar).

Remaining known headroom (~2-4%, unimplemented):
  - Variable-length edge chunks (8-step fill / 24-step drain) to
    shrink the pipeline fill+drain (~57 us).
  - Intra-chunk layer lag with unfused L1 activations, removing the
    1-chunk pipeline lag entirely (~100 us, large rewrite).
"""

import sys

sys.path.insert(0, "/opt/trn_rl_repo")

import numpy as np

B_FULL = 64
L = 2048
H = 256
W = 6
IN = 24
NCORES = 8
B = B_FULL // NCORES  # per-core batch
TC = 32  # steps per chunk

_CACHE = {}
STEP_BUFS = 1


def _build(l=L, tc_steps=TC, num_devices=NCORES, unroll_py=False, body_chunks=2,
           hints=False):
    import concourse.bass as bass
    import concourse.mybir as mybir
    import concourse.tile as tile
    from concourse import bacc
    from concourse.bass import ds
    from concourse.tile_rust import add_dep_helper

    STEP_BUFS = globals()['STEP_BUFS']
    f32 = mybir.dt.float32
    bf16 = mybir.dt.bfloat16
    AF = mybir.ActivationFunctionType

    L = l  # noqa: shadow module constants inside the builder
    TC = tc_steps
    nc = bacc.Bacc(
        "TRN2",
        target_bir_lowering=False,
        debug=False,
        enable_asserts=False,
        num_devices=num_devices,
    )

    # ---- DRAM I/O ----
    x_sl = nc.dram_tensor("x_sl", [B, L, 2], f32, kind="ExternalInput")
    w_in = {}
    for nm, shp in [
        ("Wf0", [IN, H]), ("Uf0", [H, H]), ("bf0", [H]),
        ("Wh0", [IN, H]), ("Uh0", [H, H]), ("bh0", [H]),
        ("Wf1", [H, H]), ("Uf1", [H, H]), ("bf1", [H]),
        ("Wh1", [H, H]), ("Uh1", [H, H]), ("bh1", [H]),
        ("W_out", [H + 2, 2]), ("b_out", [2]),
    ]:
        w_in[nm] = nc.dram_tensor(nm, shp, f32, kind="ExternalInput")
    out_c = nc.dram_tensor("out_c", [2, L, B], f32, kind="ExternalOutput")

    with tile.TileContext(nc) as tc:
        import contextlib

        with contextlib.ExitStack() as ctx:
            singles = ctx.enter_context(tc.tile_pool(name="singles", bufs=1))
            ftmp = ctx.enter_context(tc.tile_pool(name="ftmp", bufs=1))
            xwin_p = ctx.enter_context(tc.tile_pool(name="xwin", bufs=1))
            # Chunk-sized step intermediates (indexed by t) instead of small
            # rotating tiles: every step writes fresh addresses, so consumers
            # carry a single RAW semaphore wait — no write-after-read waits,
            # which would otherwise lower to SEQ-blocking EventSemaphore
            # instructions (~0.9us of sequencer occupancy per step).
            step_p = ctx.enter_context(tc.tile_pool(name="step", bufs=STEP_BUFS))
            head_p = ctx.enter_context(tc.tile_pool(name="head", bufs=2))
            psf_p = ctx.enter_context(tc.tile_pool(name="psf", bufs=1, space="PSUM"))
            psh_p = ctx.enter_context(tc.tile_pool(name="psh", bufs=1, space="PSUM"))
            pshd_p = ctx.enter_context(tc.tile_pool(name="pshd", bufs=2, space="PSUM"))

            # ---------- weights into lhsT layouts (bf16) ----------
            # U-style: [k(part), kt, mt, m]
            U_sb = {}
            for nm in ["Uf0", "Uh0", "Uf1", "Uh1", "Wf1", "Wh1"]:
                stage = ftmp.tile([128, 2, 2, 128], f32, tag="wstage")
                nc.sync.dma_start(
                    out=stage,
                    in_=w_in[nm].ap().rearrange(
                        "(kt k) (mt m) -> k kt mt m", kt=2, mt=2
                    ),
                )
                t = singles.tile([128, 2, 2, 128], bf16, tag=f"w_{nm}")
                nc.vector.tensor_copy(t, stage)
                U_sb[nm] = t
            # W0 with bias row: [25, mt, m], bf16
            W0_sb = {}
            for nm, bnm in [("Wf0", "bf0"), ("Wh0", "bh0")]:
                stage = ftmp.tile([IN + 1, 2, 128], f32, tag=f"wstage0_{nm}")
                nc.sync.dma_start(
                    out=stage[0:IN],
                    in_=w_in[nm].ap().rearrange("k (mt m) -> k mt m", mt=2),
                )
                nc.sync.dma_start(
                    out=stage[IN : IN + 1],
                    in_=w_in[bnm].ap().rearrange("(o mt m) -> o mt m", o=1, mt=2),
                )
                t = singles.tile([IN + 1, 2, 128], bf16, tag=f"w_{nm}")
                nc.vector.tensor_copy(t, stage)
                W0_sb[nm] = t
            # layer-1 bias rows: [1, mt, m] bf16
            b1_sb = {}
            for bnm in ["bf1", "bh1"]:
                stage = ftmp.tile([1, 2, 128], f32, tag=f"bstage_{bnm}")
                nc.sync.dma_start(
                    out=stage,
                    in_=w_in[bnm].ap().rearrange("(o mt m) -> o mt m", o=1, mt=2),
                )
                t = singles.tile([1, 2, 128], bf16, tag=f"w_{bnm}")
                nc.vector.tensor_copy(t, stage)
                b1_sb[bnm] = t
            # head weights
            wo_stage = ftmp.tile([128, 2, 2], f32, tag="wo_stage")
            nc.sync.dma_start(
                out=wo_stage,
                in_=w_in["W_out"].ap()[0:H].rearrange("(kt k) c -> k kt c", kt=2),
            )
            Wout_sb = singles.tile([128, 2, 2], bf16, tag="w_out")
            nc.vector.tensor_copy(Wout_sb, wo_stage)
            Wout_sc = singles.tile([2, 2], f32, tag="w_out_sc")
            nc.sync.dma_start(out=Wout_sc, in_=w_in["W_out"].ap()[H : H + 2])
            Wout_b = singles.tile([1, 2], f32, tag="w_out_b")
            nc.sync.dma_start(
                out=Wout_b, in_=w_in["b_out"].ap().rearrange("(o c) -> o c", o=1)
            )

            # ---------- features ----------
            PF = min(128, L)
            TL = L // PF
            xt = ftmp.tile([PF, B, TL, 2], f32, tag="xt")
            nc.sync.dma_start(
                out=xt,
                in_=x_sl.ap().rearrange("b (p tl) c -> p b tl c", p=PF),
            )
            i_v = xt[:, :, :, 0]
            q_v = xt[:, :, :, 1]
            amp2 = ftmp.tile([PF, B, TL], f32, tag="amp2")
            qq = ftmp.tile([PF, B, TL], f32, tag="qq")
            nc.vector.tensor_mul(amp2, i_v, i_v)
            nc.vector.tensor_mul(qq, q_v, q_v)
            nc.vector.tensor_add(amp2, amp2, qq)
            eps_b = ftmp.tile([PF, 1], f32, tag="eps_b")
            nc.vector.memset(eps_b, 1e-8)
            zero_b2 = ftmp.tile([PF, 1], f32, tag="zero_b2")
            nc.vector.memset(zero_b2, 0.0)
            zero_b = ftmp.tile([128, 1], f32, tag="zero_b")
            nc.vector.memset(zero_b, 0.0)
            amp = ftmp.tile([PF, B, TL], f32, tag="amp")
            nc.scalar.activation(amp, amp2, AF.Sqrt, bias=zero_b2)
            amp3 = ftmp.tile([PF, B, TL], f32, tag="amp3")
            nc.vector.tensor_mul(amp3, amp, amp2)
            seps = ftmp.tile([PF, B, TL], f32, tag="seps")
            nc.scalar.activation(seps, amp2, AF.Sqrt, bias=eps_b)
            rr = ftmp.tile([PF, B, TL], f32, tag="rr")
            nc.vector.reciprocal(rr, seps)
            sinp = ftmp.tile([PF, B, TL], f32, tag="sinp")
            cosp = ftmp.tile([PF, B, TL], f32, tag="cosp")
            nc.vector.tensor_mul(sinp, q_v, rr)
            nc.vector.tensor_mul(cosp, i_v, rr)

            # Preamble bulk DMAs rotate across three engine DMA queues so
            # their ~0.6us/issue sequencer cost parallelizes.
            _dmaq = [nc.sync, nc.scalar, nc.gpsimd]
            _dmaqi = [0]

            def dmaq():
                _dmaqi[0] += 1
                return _dmaq[_dmaqi[0] % len(_dmaq)]

            # sincosT [2, b, t] persistent (f32, head); ones tiles for bias rhs
            # Its row-build DMAs are only needed by the first head (~2 chunks
            # in), so they are emitted AFTER the fill chunk, on the SP queue
            # (idle during the fill), keeping them off the xwin critical path.
            sincosT = singles.tile([2, B, L], f32, tag="sincosT")

            def emit_sincos_dmas():
                for c, src in [(0, sinp), (1, cosp)]:
                    for b in range(B):
                        nc.sync.dma_start(
                            out=sincosT[c : c + 1, b, :].rearrange(
                                "o (p tl) -> o p tl", p=PF
                            ),
                            in_=src[:, b, :],
                        )
            ones_f = singles.tile([1, TC * B], f32, tag="ones_f")
            nc.vector.memset(ones_f, 1.0)
            ones_bf = singles.tile([1, TC * B], bf16, tag="ones_bf")
            nc.vector.memset(ones_bf, 1.0)

            # bf16 feature planes
            planes_bf = []
            for pname, src in [("ib", i_v), ("qb", q_v), ("ampb", amp), ("a3b", amp3)]:
                t = ftmp.tile([PF, B, TL], bf16, tag=pname)
                nc.vector.tensor_copy(t, src)
                planes_bf.append(t)

            # xwinT [25, b, t] bf16: row w*4+c at time t = feats_c[(t + w - 5) % L]
            # rows 20..23 are the delta=0 taps; row 24 all-ones (bias row)
            xwinT = xwin_p.tile([IN + 1, B, L], bf16, tag="xwinT")
            nc.vector.memset(xwinT, 1.0)
            for c, src in enumerate(planes_bf):
                r = (W - 1) * 4 + c
                for b in range(B):
                    dmaq().dma_start(
                        out=xwinT[r : r + 1, b, :].rearrange(
                            "o (p tl) -> o p tl", p=PF
                        ),
                        in_=src[:, b, :],
                    )
            for w in range(W - 1):
                d = w - (W - 1)  # -5 .. -1
                for c in range(4):
                    r = w * 4 + c
                    rsrc = (W - 1) * 4 + c
                    dmaq().dma_start(
                        out=xwinT[r : r + 1, :, -d:L],
                        in_=xwinT[rsrc : rsrc + 1, :, 0 : L + d],
                    )
                    dmaq().dma_start(
                        out=xwinT[r : r + 1, :, 0:-d],
                        in_=xwinT[rsrc : rsrc + 1, :, L + d : L],
                    )

            # carries + ping-pong history tiles (both layers fused)
            hcar = singles.tile([128, 2, 2, B], bf16, tag="hcar")
            nc.vector.memset(hcar, 0.0)
            histA = singles.tile([128, 2, 2, TC, B], bf16, tag="histA")
            histB = singles.tile([128, 2, 2, TC, B], bf16, tag="histB")

            def do_head(hist_w, ivh):
                ps_hd = pshd_p.tile([2, TC, B], f32, tag="pshd")
                for kt in range(2):
                    nc.tensor.matmul(
                        ps_hd.rearrange("p t b -> p (t b)"),
                        Wout_sb[:, kt],
                        hist_w[:, 1, kt, :, :].rearrange("p t b -> p (t b)"),
                        start=(kt == 0),
                        stop=False,
                    )
                nc.tensor.matmul(
                    ps_hd.rearrange("p t b -> p (t b)"),
                    Wout_sc,
                    sincosT[:, :, ds(ivh, TC)].rearrange("k b t -> k t b"),
                    start=False,
                    stop=False,
                )
                nc.tensor.matmul(
                    ps_hd.rearrange("p t b -> p (t b)"),
                    Wout_b,
                    ones_f,
                    start=False,
                    stop=True,
                )
                # copy on DVE, not ACT: at chunk boundaries an ACT-side copy
                # would queue ahead of the next chunk's sigmoid.
                head_sb = head_p.tile([2, TC, B], f32, tag="head_sb")
                nc.vector.tensor_copy(head_sb, ps_hd)
                nc.sync.dma_start(out=out_c.ap()[:, ds(ivh, TC), :], in_=head_sb)

            def scan_pair(l0_iv, l1_iv, hist_w, hist_r, dm_prev=None,
                          head_prev=None, head_self=False, prev_acts=None):
                """One fused chunk: layer0 at chunk l0_iv, layer1 at l1_iv.

                hist_w: fused history tile written this chunk (layer0 part is
                read as layer1's input next chunk; layer1 part feeds the head).
                hist_r: previous chunk's history (layer1's x-side input).
                Either layer may be inactive (iv=None) for pipeline fill/drain.
                dm_prev: unused legacy knob (the t=0 carry always flows
                through hcar; cross-chunk step-tile reads are not ordered by
                the tile framework and were verified racy on hardware).

                The x-side pre-activations are emitted in three slices: the
                t=0 and t=1 columns first (tiny), then the t>=2 bulk AFTER
                step 1's instructions — so the ~1.7us of bulk matmul exec
                runs in PE idle windows instead of stalling step 0/1's chain
                at the chunk boundary.
                """
                layers = [ly for ly, iv in ((0, l0_iv), (1, l1_iv)) if iv is not None]
                LS = slice(layers[0], layers[-1] + 1)
                psf = psf_p.tile([128, 2, 2, TC, B], f32, tag="psf")
                psh = psh_p.tile([128, 2, 2, TC, B], f32, tag="psh")

                bank_first = {}

                def open_bank(mm_ins, ps):
                    """A start=True matmul resets accumulation state beyond
                    its own address range, so it must not run before the
                    PREVIOUS chunk's last reader of this PSUM tile (prev_acts
                    = its last sigmoid/tanh). Range-based WAR tracking alone
                    does not see this hazard. Across the For_i back edge /
                    strict barriers prev_acts is None (barrier covers it)."""
                    if prev_acts is not None:
                        dep = prev_acts[0] if ps is psf else prev_acts[1]
                        if dep is not None:
                            add_dep_helper(mm_ins, dep, True, "psum bank WAR")

                def xside(ts0, ts1, close=False):
                    """Emit x-side mms covering chunk-local steps [ts0, ts1).

                    close=True marks the last mm per bank with stop=True,
                    formally closing the PSUM accumulation group (the later
                    per-step recurrent mms accumulate with skip_group_check,
                    mirroring the proven v1 flag pattern)."""
                    n = ts1 - ts0
                    for ps, g0w, g1w, g1b in (
                        (psf, "Wf0", "Wf1", "bf1"),
                        (psh, "Wh0", "Wh1", "bh1"),
                    ):
                        for ly in layers:
                            bk = (id(ps), ly)
                            if ly == 0:
                                rhs = xwinT[:, :, ds(l0_iv + ts0, n)].rearrange(
                                    "k b t -> k t b"
                                )
                                for mt in range(2):
                                    mm = nc.tensor.matmul(
                                        ps[:, 0, mt, ts0:ts1].rearrange(
                                            "p t b -> p (t b)"
                                        ),
                                        W0_sb[g0w][:, mt],
                                        rhs,
                                        start=(bk not in bank_first),
                                        stop=(ts1 == TC and mt == 1),
                                        skip_group_check=(bk in bank_first),
                                    )
                                    if bk not in bank_first:
                                        bank_first[bk] = mm.ins
                                        open_bank(mm.ins, ps)
                                    else:
                                        add_dep_helper(
                                            mm.ins, bank_first[bk], False,
                                            "bank start order",
                                        )
                            else:
                                for mt in range(2):
                                    for kt in range(2):
                                        mm = nc.tensor.matmul(
                                            ps[:, 1, mt, ts0:ts1].rearrange(
                                                "p t b -> p (t b)"
                                            ),
                                            U_sb[g1w][:, kt, mt],
                                            hist_r[:, 0, kt, ts0:ts1, :].rearrange(
                                                "p t b -> p (t b)"
                                            ),
                                            start=(bk not in bank_first),
                                            stop=False,
                                            skip_group_check=(bk in bank_first),
                                        )
                                        if bk not in bank_first:
                                            bank_first[bk] = mm.ins
                                            open_bank(mm.ins, ps)
                                        else:
                                            add_dep_helper(
                                                mm.ins, bank_first[bk], False,
                                                "bank start order",
                                            )
                                    mm = nc.tensor.matmul(
                                        ps[:, 1, mt, ts0:ts1].rearrange(
                                            "p t b -> p (t b)"
                                        ),
                                        b1_sb[g1b][:, mt],
                                        ones_bf[:, 0 : n * B],
                                        start=False,
                                        stop=(ts1 == TC and mt == 1),
                                        skip_group_check=True,
                                    )
                                    add_dep_helper(
                                        mm.ins, bank_first[bk], False,
                                        "bank start order",
                                    )

                # ---- the fused per-step scan ----
                # step 0's recurrent matmuls are emitted BEFORE the x-side
                # columns and carry the PSUM start flag for their banks, so
                # the next chunk's sigmoid isn't queued behind x-side work.
                # All step intermediates are chunk-sized, indexed by t
                # (pool bufs=1: the same buffers every chunk, so step 0 can
                # read the previous chunk's d/m at index TC-1 directly).
                f_a = step_p.tile([128, 2, 2, TC, B], bf16, tag="f")
                g_a = step_p.tile([128, 2, 2, TC, B], bf16, tag="g")
                d_a = step_p.tile([128, 2, 2, TC, B], bf16, tag="d")
                m_a = step_p.tile([128, 2, 2, TC, B], bf16, tag="m")
                th_a = step_p.tile([128, 2, 2, TC, B], bf16, tag="th")
                # bulk x-side (t>=2) emitted in small spans across steps 2..7
                # so each piece fits a PE idle window instead of queueing a
                # ~1.4us matmul block ahead of the step chain.
                NSPAN = 6
                spans = []
                lo = 2
                for k in range(NSPAN):
                    hi = min(2 + ((TC - 2) * (k + 1) + NSPAN - 1) // NSPAN, TC)
                    if lo < hi:
                        spans.append((lo, hi))
                    lo = hi
                for t in range(TC):
                    if 2 <= t < 2 + len(spans):
                        xside(*spans[t - 2])
                    if t == 9 and head_prev is not None:
                        do_head(*head_prev)
                    hprev = hcar[:, LS] if t == 0 else hist_w[:, LS, :, t - 1, :]
                    # f-gate recurrent matmuls (split: U@d + U@m for t>0).
                    # At t==0 the carry comes via hcar: the previous chunk's
                    # d/m live in a DIFFERENT pool-tile allocation, and reads
                    # of a fresh allocation's memory are not ordered against
                    # the old allocation's writers (verified racy on HW).
                    if t == 0:
                        def srcs_of(ly):
                            return [(hcar, None)]
                    else:
                        def srcs_of(ly):
                            return [(d_a, t - 1), (m_a, t - 1)]
                    for si in range(2):
                        for ly in layers:
                            sl = srcs_of(ly)
                            if si >= len(sl):
                                continue
                            src, ti = sl[si]
                            Uf = U_sb["Uf0" if ly == 0 else "Uf1"]
                            bk = (id(psf), ly)
                            for mt in range(2):
                                for kt in range(2):
                                    rhs = (
                                        src[:, ly, kt, :]
                                        if ti is None
                                        else src[:, ly, kt, ti, :]
                                    )
                                    mm = nc.tensor.matmul(
                                        psf[:, ly, mt, t, :],
                                        Uf[:, kt, mt],
                                        rhs,
                                        start=(bk not in bank_first),
                                        stop=False,
                                        skip_group_check=(bk in bank_first),
                                    )
                                    if bk not in bank_first:
                                        bank_first[bk] = mm.ins
                                        open_bank(mm.ins, psf)
                    if t == 0:
                        xside(0, 1)
                        xside(1, 2)
                    sig_i = nc.scalar.activation(
                        f_a[:, LS, :, t, :], psf[:, LS, :, t, :], AF.Sigmoid,
                        bias=zero_b,
                    )
                    nc.vector.tensor_mul(
                        g_a[:, LS, :, t, :], f_a[:, LS, :, t, :], hprev
                    )
                    nc.vector.tensor_sub(
                        d_a[:, LS, :, t, :], hprev, g_a[:, LS, :, t, :]
                    )
                    for ly in layers:
                        Uh = U_sb["Uh0" if ly == 0 else "Uh1"]
                        for mt in range(2):
                            for kt in range(2):
                                nc.tensor.matmul(
                                    psh[:, ly, mt, t, :],
                                    Uh[:, kt, mt],
                                    g_a[:, ly, kt, t, :],
                                    start=False,
                                    stop=False,
                                    skip_group_check=True,
                                )
                    tnh_i = nc.scalar.activation(
                        th_a[:, LS, :, t, :], psh[:, LS, :, t, :], AF.Tanh,
                        bias=zero_b,
                    )
                    nc.vector.tensor_mul(
                        m_a[:, LS, :, t, :], f_a[:, LS, :, t, :],
                        th_a[:, LS, :, t, :],
                    )
                    # at the chunk's last step the hcar carry is emitted
                    # FIRST: the next chunk's step-0 matmuls wait on hcar,
                    # while the hist write only feeds the (deferred) head and
                    # the next chunk's x-side bulk.
                    if t == TC - 1:
                        nc.vector.tensor_add(
                            hcar[:, LS], d_a[:, LS, :, t, :], m_a[:, LS, :, t, :]
                        )
                    nc.vector.tensor_add(
                        hist_w[:, LS, :, t, :], d_a[:, LS, :, t, :],
                        m_a[:, LS, :, t, :],
                    )

                # this chunk's head is deferred into the next chunk's step
                # loop (PE/DVE idle windows) via the caller's head_prev token;
                # only the pipeline-drain chunk emits its own head directly.
                if head_self:
                    do_head(hist_w, l1_iv)
                # last readers of psf/psh: the next chunk's bank-opening
                # matmuls must be ordered after these (see open_bank).
                return (sig_i.ins, tnh_i.ins)

            # pipeline: fill with layer0 chunk 0, then fused chunk pairs
            # (ping-pong through histA/histB), then drain layer1's last chunk.
            # chunk c writes histA when c is even, histB when odd.
            def hw_of(c):
                return histA if c % 2 == 0 else histB

            NCH = L // TC
            nb = body_chunks
            assert nb % 2 == 0, "body_chunks must be even (hist ping-pong parity)"

            def tok(c):
                # head token for the chunk-index-c scan (which ran L1 chunk
                # c-1); None when that scan had no L1 part.
                return (hw_of(c), (c - 1) * TC) if c >= 1 else None

            # `acts` threads the previous chunk's last PSUM readers into the
            # next chunk's bank-opening matmuls (see open_bank); it is reset
            # to None across strict barriers / the For_i back edge, which
            # already order everything. `started` is the cold-start marker.
            acts = scan_pair(0, None, histA, None)
            started = True
            emit_sincos_dmas()
            # peel the first two fused chunks so the in-loop head token
            # offset iv - 2*TC always refers to a valid chunk.
            pre = [c for c in (1, 2) if c <= NCH - 1]
            for c in pre:
                acts = scan_pair(c * TC, (c - 1) * TC, hw_of(c), hw_of(c - 1),
                                 dm_prev=started, head_prev=tok(c - 1),
                                 prev_acts=acts)
            tc.strict_bb_all_engine_barrier()
            first_c = (pre[-1] + 1) if pre else 1
            npl = NCH - first_c  # fused pairs left for loop + rem peels
            K = npl // nb
            rem = npl - K * nb
            if K > 0:
                import concourse.mybir as _mybir

                hint_engines = tuple(_mybir.ALL_ENGINES) if hints else ()
                if unroll_py:
                    ivs = range(first_c * TC, (first_c + K * nb) * TC, nb * TC)
                    for iv in ivs:
                        acts = None
                        for j in range(nb):
                            c = iv // TC + j
                            acts = scan_pair(
                                iv + j * TC, iv + (j - 1) * TC,
                                hw_of(c), hw_of(c - 1), dm_prev=started,
                                head_prev=(hw_of(c - 1), iv + (j - 2) * TC),
                                prev_acts=acts,
                            )
                else:
                    with tc.For_i(
                        first_c * TC, (first_c + K * nb) * TC, nb * TC,
                        hint_engines=hint_engines,
                    ) as iv:
                        acts = None
                        for j in range(nb):
                            c = first_c + j  # parity only (nb is even)
                            acts = scan_pair(
                                iv + j * TC, iv + (j - 1) * TC,
                                hw_of(c), hw_of(c - 1), dm_prev=started,
                                head_prev=(hw_of(c - 1), iv + (j - 2) * TC),
                                prev_acts=acts,
                            )
            tc.strict_bb_all_engine_barrier()
            acts = None
            for c in range(NCH - rem, NCH):
                acts = scan_pair(c * TC, (c - 1) * TC, hw_of(c), hw_of(c - 1),
                                 dm_prev=started, head_prev=tok(c - 1),
                                 prev_acts=acts)
            scan_pair(None, (NCH - 1) * TC, hw_of(NCH), hw_of(NCH - 1),
                      dm_prev=started, head_prev=tok(NCH - 1), head_self=True,
                      prev_acts=acts)

    nc.compile()
    return nc


def _get_nc():
    if "nc" not in _CACHE:
        _CACHE["nc"] = _build()
    return _CACHE["nc"]


def kernel(**inputs):
    from concourse.bass_utils import run_bass_kernel_spmd

    nc = _get_nc()
    wnames = [
        "Wf0", "Uf0", "bf0", "Wh0", "Uh0", "bh0",
        "Wf1", "Uf1", "bf1", "Wh1", "Uh1", "bh1",
        "W_out", "b_out",
    ]
    x = np.asarray(inputs["x"], dtype=np.float32)
    in_maps = []
    for c in range(NCORES):
        m = {"x_sl": np.ascontiguousarray(x[c * B : (c + 1) * B])}
        for nm in wnames:
            m[nm] = np.asarray(inputs[nm], dtype=np.float32)
        in_maps.append(m)
    res = run_bass_kernel_spmd(nc, in_maps, list(range(NCORES)))
    out = np.empty((B_FULL, L, 2), np.float32)
    for c in range(NCORES):
        oc = res.results[c]["out_c"]  # [2, L, B]
        out[c * B : (c + 1) * B] = oc.transpose(2, 1, 0)
    return out


if __name__ == "__main__":
    print("building...")
    _get_nc()
    print("built ok")



# revision 13
# speedup vs baseline: 3.6197x; 3.6197x over previous
"""Trainium2 Bass kernel for the 2-layer sMGU RNN (nn_RVTDSMGU).

v3: SEGMENT-PARALLEL scan. The MGU recurrence contracts fast (state error
from a cold h=0 restart decays ~10x per 8 steps; 60 warmup steps reach
fp32 noise — measured in numpy against the exact scan). So each sample's
L=2048 sequence is split into NSEG=8 segments of 256 steps, each segment
computed as an independent chain: 60 warmup steps (discarded) + 256
output steps. All 8 samples x 8 segments = C=64 chains run as columns of
the same per-step instructions, so the serial step count drops from 2048
to NSTEPS=316 while the per-step engine-hop latency (the floor: PE accum
-> sigmoid -> mul -> PE accum -> tanh -> mul, ~1.7-2.3us) is paid 6.5x
fewer times.

Segment 0 has no preceding context: its 60 warmup columns are ZEROS, and
with zero x-side (including the bias row) the MGU fixed point from h=0 is
exactly h=0 (f=sigmoid(0)=0.5, htilde=tanh(0)=0, h'=0.5*0+0.5*0=0), so
segment 0 starts its real steps from the true initial condition exactly.
Segments s>=1 warm up on the real columns t in [s*256-60, s*256).

Carried over from v2 (see kernel_v2_baseline.py.bak for the full notes):
  - fused layer0/layer1 step chain, layer1 one chunk behind layer0;
  - d/m split (Uf@h' = Uf@d + Uf@m) so the next step's f-gate matmuls
    wait only on m = f*tanh;
  - chunk-sized t-indexed step tiles (bufs=1) to avoid WAR semaphores;
  - PSUM bank-opening deps (open_bank) across chunks;
  - x-side pre-activations emitted in small spans inside the step loop.

Chunking: TC=4 steps/chunk, NCH=79 chunks (15 warmup chunks + 64 output
chunks). PSUM: psf/psh [128,2,2,4,64] f32 = 2 banks each, head 2 banks.

Sharding: data-parallel over batch, B=64 -> 8 cores x 8 samples.
"""

import sys

sys.path.insert(0, "/opt/trn_rl_repo")

import numpy as np

B_FULL = 64
L = 2048
H = 256
W = 6
IN = 24
NCORES = 8
B = B_FULL // NCORES  # per-core samples
NSEG = 8              # sequence segments per sample
SEG = L // NSEG       # output steps per segment
WU = 60               # warmup steps per segment
C = B * NSEG          # chains (matmul columns) per core
TC = 4                # steps per chunk
NSTEPS = SEG + WU     # serial steps
NCH = NSTEPS // TC    # chunks
WUCH = WU // TC       # warmup chunks (no head)

assert NSTEPS % TC == 0 and WU % TC == 0

_CACHE = {}


def _build(reps=1):
    import concourse.bass as bass
    import concourse.mybir as mybir
    import concourse.tile as tile
    from concourse import bacc
    from concourse.bass import ds
    from concourse.tile_rust import add_dep_helper

    f32 = mybir.dt.float32
    bf16 = mybir.dt.bfloat16
    AF = mybir.ActivationFunctionType

    nc = bacc.Bacc(
        "TRN2",
        target_bir_lowering=False,
        debug=False,
        enable_asserts=False,
        num_devices=NCORES,
    )

    # ---- DRAM I/O ----
    x_sl = nc.dram_tensor("x_sl", [B, L, 2], f32, kind="ExternalInput")
    w_in = {}
    for nm, shp in [
        ("Wf0", [IN, H]), ("Uf0", [H, H]), ("bf0", [H]),
        ("Wh0", [IN, H]), ("Uh0", [H, H]), ("bh0", [H]),
        ("Wf1", [H, H]), ("Uf1", [H, H]), ("bf1", [H]),
        ("Wh1", [H, H]), ("Uh1", [H, H]), ("bh1", [H]),
        ("W_out", [H + 2, 2]), ("b_out", [2]),
    ]:
        w_in[nm] = nc.dram_tensor(nm, shp, f32, kind="ExternalInput")
    out_c = nc.dram_tensor("out_c", [2, L, B], f32, kind="ExternalOutput")

    with tile.TileContext(nc) as tc:
        import contextlib

        with contextlib.ExitStack() as ctx:
            singles = ctx.enter_context(tc.tile_pool(name="singles", bufs=1))
            ftmp = ctx.enter_context(tc.tile_pool(name="ftmp", bufs=1))
            xwin_p = ctx.enter_context(tc.tile_pool(name="xwin", bufs=1))
            stage_ctx = contextlib.ExitStack()
            stage_p = stage_ctx.enter_context(
                tc.tile_pool(name="stage", bufs=1)
            )

            # ---------- weights into lhsT layouts (bf16) ----------
            U_sb = {}
            for nm in ["Uf0", "Uh0", "Uf1", "Uh1", "Wf1", "Wh1"]:
                stage = ftmp.tile([128, 2, 2, 128], f32, tag="wstage")
                nc.sync.dma_start(
                    out=stage,
                    in_=w_in[nm].ap().rearrange(
                        "(kt k) (mt m) -> k kt mt m", kt=2, mt=2
                    ),
                )
                t = singles.tile([128, 2, 2, 128], bf16, tag=f"w_{nm}")
                nc.vector.tensor_copy(t, stage)
                U_sb[nm] = t
            W0_sb = {}
            for nm, bnm in [("Wf0", "bf0"), ("Wh0", "bh0")]:
                stage = ftmp.tile([IN + 1, 2, 128], f32, tag=f"wstage0_{nm}")
                nc.sync.dma_start(
                    out=stage[0:IN],
                    in_=w_in[nm].ap().rearrange("k (mt m) -> k mt m", mt=2),
                )
                nc.sync.dma_start(
                    out=stage[IN : IN + 1],
                    in_=w_in[bnm].ap().rearrange("(o mt m) -> o mt m", o=1, mt=2),
                )
                t = singles.tile([IN + 1, 2, 128], bf16, tag=f"w_{nm}")
                nc.vector.tensor_copy(t, stage)
                W0_sb[nm] = t
            b1_sb = {}
            for bnm in ["bf1", "bh1"]:
                stage = ftmp.tile([1, 2, 128], f32, tag=f"bstage_{bnm}")
                nc.sync.dma_start(
                    out=stage,
                    in_=w_in[bnm].ap().rearrange("(o mt m) -> o mt m", o=1, mt=2),
                )
                t = singles.tile([1, 2, 128], bf16, tag=f"w_{bnm}")
                nc.vector.tensor_copy(t, stage)
                b1_sb[bnm] = t
            wo_stage = ftmp.tile([128, 2, 2], f32, tag="wo_stage")
            nc.sync.dma_start(
                out=wo_stage,
                in_=w_in["W_out"].ap()[0:H].rearrange("(kt k) c -> k kt c", kt=2),
            )
            Wout_sb = singles.tile([128, 2, 2], bf16, tag="w_out")
            nc.vector.tensor_copy(Wout_sb, wo_stage)
            wsc_stage = ftmp.tile([2, 2], f32, tag="w_out_sc_stage")
            nc.sync.dma_start(out=wsc_stage, in_=w_in["W_out"].ap()[H : H + 2])
            Wout_sc = singles.tile([2, 2], bf16, tag="w_out_sc")
            nc.vector.tensor_copy(Wout_sc, wsc_stage)
            Wout_b = singles.tile([1, 2], f32, tag="w_out_b")
            nc.sync.dma_start(
                out=Wout_b, in_=w_in["b_out"].ap().rearrange("(o c) -> o c", o=1)
            )

            # ---------- features ----------
            PF = 128
            TL = L // PF
            xt = ftmp.tile([PF, B, TL, 2], f32, tag="xt")
            nc.sync.dma_start(
                out=xt,
                in_=x_sl.ap().rearrange("b (p tl) c -> p b tl c", p=PF),
            )
            i_v = xt[:, :, :, 0]
            q_v = xt[:, :, :, 1]
            amp2 = ftmp.tile([PF, B, TL], f32, tag="amp2")
            qq = ftmp.tile([PF, B, TL], f32, tag="qq")
            nc.vector.tensor_mul(amp2, i_v, i_v)
            nc.vector.tensor_mul(qq, q_v, q_v)
            nc.vector.tensor_add(amp2, amp2, qq)
            eps_b = ftmp.tile([PF, 1], f32, tag="eps_b")
            nc.vector.memset(eps_b, 1e-8)
            zero_b2 = ftmp.tile([PF, 1], f32, tag="zero_b2")
            nc.vector.memset(zero_b2, 0.0)
            zero_b = ftmp.tile([128, 1], f32, tag="zero_b")
            nc.vector.memset(zero_b, 0.0)
            amp = ftmp.tile([PF, B, TL], f32, tag="amp")
            nc.scalar.activation(amp, amp2, AF.Sqrt, bias=zero_b2)
            amp3 = ftmp.tile([PF, B, TL], f32, tag="amp3")
            nc.vector.tensor_mul(amp3, amp, amp2)
            seps = ftmp.tile([PF, B, TL], f32, tag="seps")
            nc.scalar.activation(seps, amp2, AF.Sqrt, bias=eps_b)
            rr = ftmp.tile([PF, B, TL], f32, tag="rr")
            nc.vector.reciprocal(rr, seps)
            sinp = ftmp.tile([PF, B, TL], f32, tag="sinp")
            cosp = ftmp.tile([PF, B, TL], f32, tag="cosp")
            nc.vector.tensor_mul(sinp, q_v, rr)
            nc.vector.tensor_mul(cosp, i_v, rr)

            _dmaq = [nc.sync, nc.scalar, nc.gpsimd]
            _dmaqi = [0]

            def dmaq():
                _dmaqi[0] += 1
                return _dmaq[_dmaqi[0] % len(_dmaq)]

            ones_f = singles.tile([1, TC * C], f32, tag="ones_f")
            nc.vector.memset(ones_f, 1.0)
            ones_bf = singles.tile([1, TC * C], bf16, tag="ones_bf")
            nc.vector.memset(ones_bf, 1.0)

            # bf16 feature planes (incl. sin/cos for the head)
            planes_bf = []
            for pname, src in [("ib", i_v), ("qb", q_v), ("ampb", amp), ("a3b", amp3)]:
                t = ftmp.tile([PF, B, TL], bf16, tag=pname)
                nc.vector.tensor_copy(t, src)
                planes_bf.append(t)
            sinb = ftmp.tile([PF, B, TL], bf16, tag="sinb")
            nc.vector.tensor_copy(sinb, sinp)
            cosb = ftmp.tile([PF, B, TL], bf16, tag="cosb")
            nc.vector.tensor_copy(cosb, cosp)

            # xwinT [25, b, t] bf16 (STAGING): row w*4+c at time t =
            # feats_c[(t+w-5) % L]; rows 20..23 delta=0 taps; row 24 ones.
            xwinT = stage_p.tile([IN + 1, B, L], bf16, tag="xwinT")
            nc.vector.memset(xwinT, 1.0)
            for cch, src in enumerate(planes_bf):
                r = (W - 1) * 4 + cch
                for b in range(B):
                    dmaq().dma_start(
                        out=xwinT[r : r + 1, b, :].rearrange(
                            "o (p tl) -> o p tl", p=PF
                        ),
                        in_=src[:, b, :],
                    )
            for w in range(W - 1):
                d = w - (W - 1)  # -5 .. -1
                r = w * 4
                rsrc = (W - 1) * 4
                dmaq().dma_start(
                    out=xwinT[r : r + 4, :, -d:L],
                    in_=xwinT[rsrc : rsrc + 4, :, 0 : L + d],
                )
                dmaq().dma_start(
                    out=xwinT[r : r + 4, :, 0:-d],
                    in_=xwinT[rsrc : rsrc + 4, :, L + d : L],
                )

            # sincosT [2, b, t] bf16 (STAGING)
            sincosT = stage_p.tile([2, B, L], bf16, tag="sincosT")
            for cch, src in [(0, sinb), (1, cosb)]:
                for b in range(B):
                    dmaq().dma_start(
                        out=sincosT[cch : cch + 1, b, :].rearrange(
                            "o (p tl) -> o p tl", p=PF
                        ),
                        in_=src[:, b, :],
                    )

            # xsegT [25, C=(s b), NSTEPS] bf16: chain (s,b) column j holds
            # xwin of sample b at t = s*SEG - WU + j. Segment 0's warmup
            # columns (j<WU) are zeros (exact h=0 fixed point).
            xsegT = xwin_p.tile([IN + 1, C, NSTEPS], bf16, tag="xsegT")
            nc.vector.memset(xsegT[:, 0:B, 0:WU], 0.0)
            nc.sync.dma_start(
                out=xsegT[:, 0:B, WU:NSTEPS],
                in_=xwinT[:, :, 0:SEG],
            )
            # s >= 1: per-segment slab copies (windows overlap, so one
            # rearranged view can't express the source)
            for s in range(1, NSEG):
                t0 = s * SEG - WU
                dmaq().dma_start(
                    out=xsegT[:, s * B : (s + 1) * B, :],
                    in_=xwinT[:, :, t0 : t0 + NSTEPS],
                )

            # segmented sin/cos for the head: [2, C=(s b), SEG] bf16
            sincos_seg = xwin_p.tile([2, C, SEG], bf16, tag="sincos_seg")
            for s in range(NSEG):
                dmaq().dma_start(
                    out=sincos_seg[:, s * B : (s + 1) * B, :],
                    in_=sincosT[:, :, s * SEG : (s + 1) * SEG],
                )

            # free staging space before the scan pools open
            stage_ctx.close()
            step_p = ctx.enter_context(tc.tile_pool(name="step", bufs=1))
            head_p = ctx.enter_context(tc.tile_pool(name="head", bufs=2))
            psf_p = ctx.enter_context(tc.tile_pool(name="psf", bufs=1, space="PSUM"))
            psh_p = ctx.enter_context(tc.tile_pool(name="psh", bufs=1, space="PSUM"))
            pshd_p = ctx.enter_context(tc.tile_pool(name="pshd", bufs=2, space="PSUM"))

            # carries + ping-pong history tiles (both layers fused)
            hcar = singles.tile([128, 2, 2, C], bf16, tag="hcar")
            histA = singles.tile([128, 2, 2, TC, C], bf16, tag="histA")
            histB = singles.tile([128, 2, 2, TC, C], bf16, tag="histB")

            def do_head(hist_w, tlo):
                """Head for one L1 chunk; tlo = within-segment output t base
                (0..SEG-TC). Column order (s, t, b) so the out DMA folds to
                3 dims. Writes out_c[:, s*SEG + tlo + (0..TC), b]."""
                ps_hd = pshd_p.tile([2, NSEG, TC, B], f32, tag="pshd")
                for kt in range(2):
                    nc.tensor.matmul(
                        ps_hd.rearrange("p s t b -> p (s t b)"),
                        Wout_sb[:, kt],
                        hist_w[:, 1, kt, :, :].rearrange(
                            "p t (s b) -> p s t b", s=NSEG
                        ),
                        start=(kt == 0),
                        stop=False,
                    )
                nc.tensor.matmul(
                    ps_hd.rearrange("p s t b -> p (s t b)"),
                    Wout_sc,
                    sincos_seg[:, :, ds(tlo, TC)].rearrange(
                        "k (s b) t -> k s t b", s=NSEG
                    ),
                    start=False,
                    stop=False,
                )
                nc.tensor.matmul(
                    ps_hd.rearrange("p s t b -> p (s t b)"),
                    Wout_b,
                    ones_f,
                    start=False,
                    stop=True,
                )
                head_sb = head_p.tile([2, NSEG, TC, B], f32, tag="head_sb")
                nc.vector.tensor_copy(head_sb, ps_hd)
                nc.sync.dma_start(
                    out=out_c.ap()
                    .rearrange("p (s tl) b -> p s tl b", s=NSEG)[
                        :, :, ds(tlo, TC), :
                    ],
                    in_=head_sb,
                )

            def scan_pair(l0_iv, l1_iv, hist_w, hist_r, head_prev=None,
                          head_self=False, prev_acts=None):
                """One fused chunk: layer0 at serial-j base l0_iv, layer1 at
                l1_iv (one chunk behind). head_prev: (hist_tile, tlo) token
                for the deferred head of L1 chunk two scans back."""
                layers = [ly for ly, iv in ((0, l0_iv), (1, l1_iv)) if iv is not None]
                LS = slice(layers[0], layers[-1] + 1)
                psf = psf_p.tile([128, 2, 2, TC, C], f32, tag="psf")
                psh = psh_p.tile([128, 2, 2, TC, C], f32, tag="psh")

                bank_first = {}

                def open_bank(mm_ins, ps):
                    if prev_acts is not None:
                        dep = prev_acts[0] if ps is psf else prev_acts[1]
                        if dep is not None:
                            add_dep_helper(mm_ins, dep, True, "psum bank WAR")

                def xside(ts0, ts1):
                    n = ts1 - ts0
                    for ps, g0w, g1w, g1b in (
                        (psf, "Wf0", "Wf1", "bf1"),
                        (psh, "Wh0", "Wh1", "bh1"),
                    ):
                        for ly in layers:
                            bk = (id(ps), ly)
                            if ly == 0:
                                rhs = xsegT[:, :, ds(l0_iv + ts0, n)].rearrange(
                                    "k c t -> k t c"
                                )
                                for mt in range(2):
                                    mm = nc.tensor.matmul(
                                        ps[:, 0, mt, ts0:ts1].rearrange(
                                            "p t c -> p (t c)"
                                        ),
                                        W0_sb[g0w][:, mt],
                                        rhs,
                                        start=(bk not in bank_first),
                                        stop=(ts1 == TC and mt == 1),
                                        skip_group_check=(bk in bank_first),
                                    )
                                    if bk not in bank_first:
                                        bank_first[bk] = mm.ins
                                        open_bank(mm.ins, ps)
                                    else:
                                        add_dep_helper(
                                            mm.ins, bank_first[bk], False,
                                            "bank start order",
                                        )
                            else:
                                for mt in range(2):
                                    for kt in range(2):
                                        mm = nc.tensor.matmul(
                                            ps[:, 1, mt, ts0:ts1].rearrange(
                                                "p t c -> p (t c)"
                                            ),
                                            U_sb[g1w][:, kt, mt],
                                            hist_r[:, 0, kt, ts0:ts1, :].rearrange(
                                                "p t c -> p (t c)"
                                            ),
                                            start=(bk not in bank_first),
                                            stop=False,
                                            skip_group_check=(bk in bank_first),
                                        )
                                        if bk not in bank_first:
                                            bank_first[bk] = mm.ins
                                            open_bank(mm.ins, ps)
                                        else:
                                            add_dep_helper(
                                                mm.ins, bank_first[bk], False,
                                                "bank start order",
                                            )
                                    mm = nc.tensor.matmul(
                                        ps[:, 1, mt, ts0:ts1].rearrange(
                                            "p t c -> p (t c)"
                                        ),
                                        b1_sb[g1b][:, mt],
                                        ones_bf[:, 0 : n * C],
                                        start=False,
                                        stop=(ts1 == TC and mt == 1),
                                        skip_group_check=True,
                                    )
                                    add_dep_helper(
                                        mm.ins, bank_first[bk], False,
                                        "bank start order",
                                    )

                f_a = step_p.tile([128, 2, 2, TC, C], bf16, tag="f")
                g_a = step_p.tile([128, 2, 2, TC, C], bf16, tag="g")
                d_a = step_p.tile([128, 2, 2, TC, C], bf16, tag="d")
                m_a = step_p.tile([128, 2, 2, TC, C], bf16, tag="m")
                th_a = step_p.tile([128, 2, 2, TC, C], bf16, tag="th")
                for t in range(TC):
                    if 2 <= t < TC:
                        xside(t, t + 1)
                    if t == 1 and head_prev is not None:
                        do_head(*head_prev)
                    hprev = hcar[:, LS] if t == 0 else hist_w[:, LS, :, t - 1, :]
                    if t == 0:
                        srcs = [(hcar, None)]
                    else:
                        srcs = [(d_a, t - 1), (m_a, t - 1)]
                    for src, ti in srcs:
                        for ly in layers:
                            Uf = U_sb["Uf0" if ly == 0 else "Uf1"]
                            bk = (id(psf), ly)
                            for mt in range(2):
                                for kt in range(2):
                                    rhs = (
                                        src[:, ly, kt, :]
                                        if ti is None
                                        else src[:, ly, kt, ti, :]
                                    )
                                    mm = nc.tensor.matmul(
                                        psf[:, ly, mt, t, :],
                                        Uf[:, kt, mt],
                                        rhs,
                                        start=(bk not in bank_first),
                                        stop=False,
                                        skip_group_check=(bk in bank_first),
                                    )
                                    if bk not in bank_first:
                                        bank_first[bk] = mm.ins
                                        open_bank(mm.ins, psf)
                    if t == 0:
                        xside(0, 1)
                        xside(1, 2)
                    sig_i = nc.scalar.activation(
                        f_a[:, LS, :, t, :], psf[:, LS, :, t, :], AF.Sigmoid,
                        bias=zero_b,
                    )
                    nc.vector.tensor_mul(
                        g_a[:, LS, :, t, :], f_a[:, LS, :, t, :], hprev
                    )
                    nc.vector.tensor_sub(
                        d_a[:, LS, :, t, :], hprev, g_a[:, LS, :, t, :]
                    )
                    for ly in layers:
                        Uh = U_sb["Uh0" if ly == 0 else "Uh1"]
                        for mt in range(2):
                            for kt in range(2):
                                nc.tensor.matmul(
                                    psh[:, ly, mt, t, :],
                                    Uh[:, kt, mt],
                                    g_a[:, ly, kt, t, :],
                                    start=False,
                                    stop=False,
                                    skip_group_check=True,
                                )
                    tnh_i = nc.scalar.activation(
                        th_a[:, LS, :, t, :], psh[:, LS, :, t, :], AF.Tanh,
                        bias=zero_b,
                    )
                    nc.vector.tensor_mul(
                        m_a[:, LS, :, t, :], f_a[:, LS, :, t, :],
                        th_a[:, LS, :, t, :],
                    )
                    if t == TC - 1:
                        nc.vector.tensor_add(
                            hcar[:, LS], d_a[:, LS, :, t, :], m_a[:, LS, :, t, :]
                        )
                    nc.vector.tensor_add(
                        hist_w[:, LS, :, t, :], d_a[:, LS, :, t, :],
                        m_a[:, LS, :, t, :],
                    )

                if head_self:
                    do_head(hist_w, l1_iv - WU)
                return (sig_i.ins, tnh_i.ins)

            def hw_of(c):
                return histA if c % 2 == 0 else histB

            nb = 2
            # scan-chunk u emits the deferred head for L1 chunk u-2, which
            # exists (and is past warmup) iff u-2 >= WUCH.
            FIRST_HEAD_SCAN = WUCH + 2  # 17
            assert (FIRST_HEAD_SCAN - 1) % nb == 0
            assert (NCH - FIRST_HEAD_SCAN) % nb == 0

            for rep in range(reps):
                if rep > 0:
                    tc.strict_bb_all_engine_barrier()
                nc.vector.memset(hcar, 0.0)
                # fill: layer0 chunk 0 only
                acts = scan_pair(0, None, histA, None)
                tc.strict_bb_all_engine_barrier()
                # LOOP-A: scan-chunks 1..FIRST_HEAD_SCAN-1, no heads
                with tc.For_i(1 * TC, FIRST_HEAD_SCAN * TC, nb * TC) as iv:
                    acts = None
                    for j in range(nb):
                        cpar = 1 + j
                        acts = scan_pair(
                            iv + j * TC, iv + (j - 1) * TC,
                            hw_of(cpar), hw_of(cpar - 1),
                            prev_acts=acts,
                        )
                tc.strict_bb_all_engine_barrier()
                # LOOP-B: scan-chunks FIRST_HEAD_SCAN..NCH-1, heads on
                with tc.For_i(FIRST_HEAD_SCAN * TC, NCH * TC, nb * TC) as iv:
                    acts = None
                    for j in range(nb):
                        cpar = FIRST_HEAD_SCAN + j
                        acts = scan_pair(
                            iv + j * TC, iv + (j - 1) * TC,
                            hw_of(cpar), hw_of(cpar - 1),
                            head_prev=(hw_of(cpar - 1), iv + (j - 2) * TC - WU),
                            prev_acts=acts,
                        )
                tc.strict_bb_all_engine_barrier()
                # drain: layer1's last chunk + the last two heads
                scan_pair(None, (NCH - 1) * TC, hw_of(NCH), hw_of(NCH - 1),
                          head_prev=(hw_of(NCH - 1), (NCH - 2) * TC - WU),
                          head_self=True, prev_acts=None)

    nc.compile()
    return nc


def _get_nc():
    if "nc" not in _CACHE:
        _CACHE["nc"] = _build()
    return _CACHE["nc"]


def kernel(**inputs):
    from concourse.bass_utils import run_bass_kernel_spmd

    nc = _get_nc()
    wnames = [
        "Wf0", "Uf0", "bf0", "Wh0", "Uh0", "bh0",
        "Wf1", "Uf1", "bf1", "Wh1", "Uh1", "bh1",
        "W_out", "b_out",
    ]
    x = np.asarray(inputs["x"], dtype=np.float32)
    in_maps = []
    for c in range(NCORES):
        m = {"x_sl": np.ascontiguousarray(x[c * B : (c + 1) * B])}
        for nm in wnames:
            m[nm] = np.asarray(inputs[nm], dtype=np.float32)
        in_maps.append(m)
    res = run_bass_kernel_spmd(nc, in_maps, list(range(NCORES)))
    out = np.empty((B_FULL, L, 2), np.float32)
    for c in range(NCORES):
        oc = res.results[c]["out_c"]  # [2, L, B]
        out[c * B : (c + 1) * B] = oc.transpose(2, 1, 0)
    return out


if __name__ == "__main__":
    print("building...")
    _get_nc()
    print("built ok")
